# revision 1
# baseline (speedup 1.0000x reference)
# Differential multi-head attention (dual softmax + GroupNorm + sigmoid gating)
# for Trainium2, batch-parallel across 8 NeuronCores (one batch row per core).
#
# Per-core math (batch b):
#   q = query @ Wq + bq -> per head: q1, q2, gate (each S x 64)
#   k = key   @ Wk + bk -> per head: k1, k2
#   v = values@ Wv + bv -> per head: v (S x 64)
#   attn = softmax(q1 k1^T / 8) - lam * softmax(q2 k2^T / 8)
#   out  = GroupNorm_{8 groups over d, reduced over (S, heads, d-in-group)}(attn @ v)
#   out  = out * (1 - lambda_init) * sigmoid(gate)
#
# Layout strategy: d-major ("transposed") attention: scores are computed as
# s^T (k on partitions, q free) so the attn@v contraction runs at K=128, and
# exp row-sums come free via a ones-column appended to v (M=65).  q1/q2 (and
# k1/k2) of each head live in complementary 64-partition halves of one tile,
# so the two K=64 score matmuls of a head occupy disjoint PE row-groups and
# can run concurrently.  Matmul inputs are bf16 (single-pass PE); accumulation,
# softmax normalization, GroupNorm and the final output stay fp32.
# sigmoid(x) = (tanh(x/2)+1)/2 keeps ACT in one table set (exp/tanh/square).

import numpy as np

B, S_FULL, H, D = 8, 1024, 8, 64
DM = H * D  # 512


def build_nc(S=1024):
    import concourse.bacc as bacc
    import concourse.bass as bass
    import concourse.tile as tile
    from concourse import mybir
    from concourse.masks import make_identity

    f32 = mybir.dt.float32
    bf16 = mybir.dt.bfloat16
    AF = mybir.ActivationFunctionType
    OP = mybir.AluOpType
    AX = mybir.AxisListType

    NJ = S // 128          # k/seq 128-tiles
    CH = min(512, S)       # fp32-out matmul chunk
    NN = max(1, S // CH)
    CNT = float(S * H * (D // H))  # groupnorm reduction count per group
    EPS = 1e-3
    INV = 0.125            # 1/sqrt(64)

    nc = bacc.Bacc(target_bir_lowering=False)
    q_d = nc.dram_tensor("query", [S, DM], f32, kind="ExternalInput")
    k_d = nc.dram_tensor("key", [S, DM], f32, kind="ExternalInput")
    v_d = nc.dram_tensor("values", [S, DM], f32, kind="ExternalInput")
    wq_d = nc.dram_tensor("Wq", [DM, 3 * H * D], f32, kind="ExternalInput")
    bq_d = nc.dram_tensor("bq", [3 * H * D], f32, kind="ExternalInput")
    wk_d = nc.dram_tensor("Wk", [DM, 2 * H * D], f32, kind="ExternalInput")
    bk_d = nc.dram_tensor("bk", [2 * H * D], f32, kind="ExternalInput")
    wv_d = nc.dram_tensor("Wv", [DM, H * D], f32, kind="ExternalInput")
    bv_d = nc.dram_tensor("bv", [H * D], f32, kind="ExternalInput")
    gamma_d = nc.dram_tensor("gamma", [D], f32, kind="ExternalInput")
    beta_d = nc.dram_tensor("beta", [D], f32, kind="ExternalInput")
    lam_d = nc.dram_tensor("lam", [1], f32, kind="ExternalInput")
    li_d = nc.dram_tensor("lambda_init", [1], f32, kind="ExternalInput")
    out_d = nc.dram_tensor("out", [S, DM], f32, kind="ExternalOutput")

    ts_ = nc.vector.tensor_scalar
    stt = nc.vector.scalar_tensor_tensor

    with tile.TileContext(nc) as tc:
        with tc.tile_pool(name="consts", bufs=1) as consts, \
             tc.tile_pool(name="persist", bufs=1) as persist:

            # ---------- constants ----------
            ident = consts.tile([128, 128], f32, tag="ident", name="ident")
            make_identity(nc, ident)
            ident_b = consts.tile([128, 128], bf16, tag="ident_b", name="ident_b")
            make_identity(nc, ident_b)

            # block-diagonal group matrix: IND2[d', d] = 1 iff d'//8 == d//8
            ind2 = consts.tile([64, 64], f32, tag="ind2", name="ind2")
            nc.gpsimd.memset(ind2, 1.0)
            nc.gpsimd.affine_select(
                out=ind2, in_=ind2, compare_op=OP.is_ge, fill=0.0,
                base=0, pattern=[[-8, 8], [0, 8]], channel_multiplier=1)
            nc.gpsimd.affine_select(
                out=ind2, in_=ind2, compare_op=OP.is_ge, fill=0.0,
                base=7, pattern=[[8, 8], [0, 8]], channel_multiplier=-1)

            # selectors for the r-row broadcast matmul (per pair-half)
            # SP rows: [0]=sum1(even half), [1]=sum2, [2]=sum1(odd), [3]=sum2
            # sel[half][p, x] = 1 iff (x - 64p + 128*half) in [0, 64)
            sel = []
            for half in range(2):
                s_t = consts.tile([4, 128], f32, tag=f"sel{half}", name=f"sel{half}")
                nc.gpsimd.memset(s_t, 1.0)
                nc.gpsimd.affine_select(
                    out=s_t, in_=s_t, compare_op=OP.is_ge, fill=0.0,
                    base=128 * half, pattern=[[1, 128]], channel_multiplier=-64)
                nc.gpsimd.affine_select(
                    out=s_t, in_=s_t, compare_op=OP.is_ge, fill=0.0,
                    base=63 - 128 * half, pattern=[[-1, 128]], channel_multiplier=64)
                sel.append(s_t)

            # scalar columns
            lam64 = consts.tile([64, 1], f32, tag="lam64", name="lam64")
            nc.gpsimd.dma_start(out=lam64, in_=lam_d[:].to_broadcast([64, 1]))
            li64 = consts.tile([64, 1], f32, tag="li64", name="li64")
            nc.gpsimd.dma_start(out=li64, in_=li_d[:].to_broadcast([64, 1]))
            neglam64 = consts.tile([64, 1], f32, tag="neglam64", name="neglam64")
            ts_(neglam64, lam64, -1.0, None, OP.mult)
            onelam64 = consts.tile([64, 1], f32, tag="onelam64", name="onelam64")
            ts_(onelam64, lam64, -1.0, 1.0, OP.mult, OP.add)   # 1 - lam
            halfli = consts.tile([64, 1], f32, tag="halfli", name="halfli")
            ts_(halfli, li64, -0.5, 0.5, OP.mult, OP.add)      # 0.5*(1-li)

            gamma_c = consts.tile([64, 1], f32, tag="gamma_c", name="gamma_c")
            nc.sync.dma_start(out=gamma_c, in_=gamma_d[:])
            beta_c = consts.tile([64, 1], f32, tag="beta_c", name="beta_c")
            nc.sync.dma_start(out=beta_c, in_=beta_d[:])
            bb64 = consts.tile([64, 1], f32, tag="bb64", name="bb64")
            ts_(bb64, beta_c, halfli, None, OP.mult)           # beta*0.5*(1-li)

            # v-bias columns per head and C = bv*(1-lam) fold
            bvc = consts.tile([64, 8], f32, tag="bvc", name="bvc")
            nc.sync.dma_start(
                out=bvc, in_=bv_d[:].rearrange("(h d) -> d h", d=64))
            cc = consts.tile([64, 8], f32, tag="cc", name="cc")
            ts_(cc, bvc, onelam64, None, OP.mult)

            # bias columns: per-head stacked [q1|q2] / [k1|k2] are contiguous
            # 128-element runs of bq/bk; gate needs a gathered layout.
            bqp = consts.tile([128, 8], f32, tag="bqp", name="bqp")
            nc.sync.dma_start(
                out=bqp,
                in_=bq_d[:].rearrange("(h blk) -> blk h", blk=192)[0:128, :])
            bkp = consts.tile([128, 8], f32, tag="bkp", name="bkp")
            nc.sync.dma_start(
                out=bkp,
                in_=bk_d[:].rearrange("(h blk) -> blk h", blk=128))
            bg = consts.tile([128, 4], f32, tag="bg", name="bg")
            bqv = bq_d[:].rearrange("(h blk) -> h blk", blk=192)
            for p in range(4):
                nc.sync.dma_start(out=bg[:, p:p + 1],
                                  in_=bqv[2 * p:2 * p + 2, 128:192])

            # persistent projection outputs (bf16, d-major)
            # qp/kp[h]: rows 0-63 = q1/k1 of head h, rows 64-127 = q2/k2
            qp = [persist.tile([128, S], bf16, tag=f"qp{h}", name=f"qp{h}") for h in range(8)]
            # zero-padded key tiles: kz1[h] rows 0-63 = k1 (rest 0),
            # kz2[h] rows 64-127 = k2 (rest 0) -> K=128 score matmuls
            kz1 = [persist.tile([128, S], bf16, tag=f"kz1{h}", name=f"kz1{h}") for h in range(8)]
            kz2 = [persist.tile([128, S], bf16, tag=f"kz2{h}", name=f"kz2{h}") for h in range(8)]
            for h in range(8):
                nc.vector.memset(kz1[h][64:128, :], 0.0)
                nc.vector.memset(kz2[h][0:64, :], 0.0)
            # gate stays head-pair packed: gt[p] rows 0-63 = head 2p, 64-127 = 2p+1
            gt = [persist.tile([128, S], bf16, tag=f"gt{p}", name=f"gt{p}") for p in range(4)]
            va = [persist.tile([128, 8, 65], bf16, tag=f"va{i}", name=f"va{i}") for i in range(NJ)]
            ypair = [persist.tile([128, S], f32, tag=f"yp{p}", name=f"yp{p}") for p in range(4)]
            sumcol = persist.tile([64, 16], f32, tag="sumcol", name="sumcol")

            # ---------- phase 1: load + transpose inputs (DMA only) ----------
            # fp32 DRAM -> (cast DMA) -> bf16 DRAM scratch -> (xbar transpose
            # DMA) -> x^T bf16 in SBUF, 4 tiles of (128, S) per tensor.
            GRP = min(4, NJ)
            with tc.tile_pool(name="xin", bufs=3) as xin_pool, \
                 tc.tile_pool(name="xtp", bufs=1) as xtp, \
                 tc.tile_pool(name="wload", bufs=1) as wpool, \
                 tc.tile_pool(name="ps_in", bufs=1, space="PSUM") as ps_in, \
                 tc.tile_pool(name="ps_proj", bufs=4, space="PSUM") as ps_proj:

                def transpose_input(x_dram, nm):
                    xt = [xtp.tile([128, S], bf16, tag=f"xt{nm}{c}", name=f"xt{nm}{c}")
                          for c in range(4)]
                    tp_cur = [None] * 4
                    for i in range(NJ):
                        xs = xin_pool.tile([128, DM], f32, tag="xs", name="xs")
                        nc.sync.dma_start(out=xs, in_=x_dram[128 * i:128 * (i + 1), :])
                        xq = xin_pool.tile([128, DM], bf16, tag="xin", name="xin")
                        nc.vector.tensor_copy(xq, xs)
                        if i % GRP == 0:
                            for c in range(4):
                                tp_cur[c] = ps_in.tile(
                                    [128, 128 * GRP], bf16, tag=f"tp{c}", name=f"tp{c}")
                        for c in range(4):
                            nc.tensor.transpose(
                                tp_cur[c][:, 128 * (i % GRP):128 * (i % GRP + 1)],
                                xq[:, 128 * c:128 * (c + 1)], ident_b)
                        if i % GRP == GRP - 1:
                            base = 128 * GRP * (i // GRP)
                            for c in range(4):
                                nc.vector.tensor_copy(
                                    xt[c][:, base:base + 128 * GRP], tp_cur[c])
                    return xt

                # --- query path: qp[h] then gate ---
                # (x casts issue first so the SWDGE queue isn't stuck behind
                # the 6MB of weight casts at kernel start)
                xtq = transpose_input(q_d, "q")
                wqf = [wpool.tile([128, 3 * H * D], bf16, tag=f"wqf{r}", name=f"wqf{r}") for r in range(4)]
                wkf = [wpool.tile([128, 2 * H * D], bf16, tag=f"wkf{r}", name=f"wkf{r}") for r in range(4)]
                wvf = [wpool.tile([128, H * D], bf16, tag=f"wvf{r}", name=f"wvf{r}") for r in range(4)]
                # stage fp32 weights via HWDGE (fast, parallel to the x casts
                # on the SWDGE queue), downcast on the otherwise-idle ACT
                for r in range(4):
                    wsq = wpool.tile([128, 3 * H * D], f32, tag=f"wsq{r}", name=f"wsq{r}")
                    nc.sync.dma_start(out=wsq, in_=wq_d[128 * r:128 * (r + 1), :])
                    nc.scalar.copy(wqf[r], wsq)
                for r in range(4):
                    wsk = wpool.tile([128, 2 * H * D], f32, tag=f"wsk{r}", name=f"wsk{r}")
                    nc.sync.dma_start(out=wsk, in_=wk_d[128 * r:128 * (r + 1), :])
                    nc.scalar.copy(wkf[r], wsk)
                    wsv = wpool.tile([128, H * D], f32, tag=f"wsv{r}", name=f"wsv{r}")
                    nc.sync.dma_start(out=wsv, in_=wv_d[128 * r:128 * (r + 1), :])
                    nc.scalar.copy(wvf[r], wsv)
                for h in range(8):
                    for n in range(NN):
                        ps = ps_proj.tile([128, CH], f32, tag="proj", name="proj")
                        for r in range(4):
                            nc.tensor.matmul(
                                ps, wqf[r][:, 192 * h:192 * h + 128],
                                xtq[r][:, CH * n:CH * (n + 1)],
                                start=(r == 0), stop=(r == 3))
                        nc.scalar.activation(
                            qp[h][:, CH * n:CH * (n + 1)], ps, AF.Identity,
                            bias=bqp[:, h:h + 1])
                # gate: pre-gathered pair-packed weight tiles (the 64-col
                # blocks of heads 2p/2p+1 collected by the load DMA)
                wgt = []
                for r in range(4):
                    w_t = wpool.tile([128, 512], bf16, tag=f"wg{r}", name=f"wg{r}")
                    nc.gpsimd.dma_start(
                        out=w_t,
                        in_=wq_d[128 * r:128 * (r + 1), :].rearrange(
                            "k (h blk) -> k h blk", blk=192)[:, :, 128:192])
                    wgt.append(w_t)
                for p in range(4):
                    for n in range(NN):
                        ps = ps_proj.tile([128, CH], f32, tag="proj", name="proj")
                        for r in range(4):
                            nc.tensor.matmul(
                                ps, wgt[r][:, 128 * p:128 * (p + 1)],
                                xtq[r][:, CH * n:CH * (n + 1)],
                                start=(r == 0), stop=(r == 3))
                        nc.scalar.activation(
                            gt[p][:, CH * n:CH * (n + 1)], ps, AF.Identity,
                            bias=bg[:, p:p + 1])

                # --- key path ---
                xtk = transpose_input(k_d, "k")
                for h in range(8):
                    for n in range(NN):
                        ps = ps_proj.tile([128, CH], f32, tag="proj", name="proj")
                        for r in range(4):
                            nc.tensor.matmul(
                                ps, wkf[r][:, 128 * h:128 * (h + 1)],
                                xtk[r][:, CH * n:CH * (n + 1)],
                                start=(r == 0), stop=(r == 3))
                        nc.scalar.activation(
                            kz1[h][0:64, CH * n:CH * (n + 1)], ps[0:64, :],
                            AF.Identity, bias=bkp[0:64, h:h + 1])
                        nc.scalar.activation(
                            kz2[h][64:128, CH * n:CH * (n + 1)], ps[64:128, :],
                            AF.Identity, bias=bkp[64:128, h:h + 1])

                # --- values path (q-major, interleaved into v_aug + ones) ---
                xtv = transpose_input(v_d, "v")
                for i in range(NJ):
                    ps = ps_proj.tile([128, 512], f32, tag="proj", name="proj")
                    for r in range(4):
                        nc.tensor.matmul(
                            ps, xtv[r][:, 128 * i:128 * (i + 1)], wvf[r],
                            start=(r == 0), stop=(r == 3))
                    nc.scalar.copy(
                        va[i][:, :, 0:64],
                        ps.rearrange("p (h d) -> p h d", d=64))
                    nc.gpsimd.memset(va[i][:, :, 64:65], 1.0)

                # gate tanh now (ACT is free here; result only needed at the
                # very end) -- th_t lives in the persist pool
                th_t = [persist.tile([128, S], f32, tag=f"th{p}", name=f"th{p}")
                        for p in range(4)]
                for p in range(4):
                    nc.scalar.activation(th_t[p], gt[p], AF.Tanh, scale=0.5)

            # ---------- phase 2: attention per head (pairs for epilogue) ----
            with tc.tile_pool(name="ps_att", bufs=2, space="PSUM") as ps_att, \
                 tc.tile_pool(name="ps_o", bufs=2, space="PSUM") as ps_o, \
                 tc.tile_pool(name="expp", bufs=3) as expp, \
                 tc.tile_pool(name="osp", bufs=2) as osp, \
                 tc.tile_pool(name="spp", bufs=2) as spp:

                def emit_combine(sp, os_t, p):
                    rp = spp.tile([4, S], f32, tag="rp", name="rp", bufs=1)
                    rscr = spp.tile([4, S], f32, tag="rscr", name="rscr", bufs=1)
                    nc.vector.reciprocal_approx_accurate(rp, sp, rscr)
                    rst = []
                    if p < 3:
                        for i in range(4):
                            r_t = spp.tile([1, S], f32, tag=f"rst{i}", name=f"rst{i}", bufs=1)
                            nc.sync.dma_start(out=r_t, in_=rp[i:i + 1, :])
                            rst.append(r_t)
                    for half in range(2):
                        h = 2 * p + half
                        os1, os2 = os_t[half]
                        if p == 3:
                            # last pair: PE idles at the tail barrier, so use
                            # the selector-matmul broadcast (lower latency,
                            # keeps HAM warm into the output transposes)
                            bc = ps_att.tile([128, S], f32, tag="s", name="bcpe")
                            for n in range(NN):
                                nc.tensor.matmul(
                                    bc[:, CH * n:CH * (n + 1)], sel[half],
                                    rp[:, CH * n:CH * (n + 1)],
                                    start=True, stop=True)
                            bcs1, bcs2 = bc[0:64, :], bc[64:128, :]
                        else:
                            bcs1 = spp.tile([64, S], f32, tag="bcs1", name="bcs1", bufs=1)
                            bcs2 = spp.tile([64, S], f32, tag="bcs2", name="bcs2", bufs=1)
                            nc.gpsimd.partition_broadcast(
                                bcs1, rst[2 * half][0:1, :], channels=64)
                            nc.gpsimd.partition_broadcast(
                                bcs2, rst[2 * half + 1][0:1, :], channels=64)
                        nc.vector.tensor_mul(os1[0:64, :], os1[0:64, :], bcs1)
                        stt(os2[0:64, :], os2[0:64, :], neglam64, bcs2,
                            OP.mult, OP.mult)
                        ydst = ypair[p][64 * half:64 * half + 64, :]
                        stt(ydst, os1[0:64, :], 1.0, os2[0:64, :],
                            OP.bypass, OP.add, accum_out=sumcol[:, h:h + 1])
                        stt(os1[0:64, :], ydst, 1.0, ydst, OP.mult, OP.mult,
                            accum_out=sumcol[:, 8 + h:9 + h])

                for p in range(4):
                    os_t = {}
                    sp = spp.tile([4, S], f32, tag="sp", name="sp")
                    for half in range(2):
                        h = 2 * p + half
                        os_c = {}
                        # term-sequential: only one o accumulator lives at a
                        # time, so both the score tiles and the o tiles can
                        # double-buffer inside the 8-bank PSUM budget.
                        for t, kz_ in ((1, kz1), (2, kz2)):
                            o_ps = ps_o.tile([65, S], f32, tag="o", name="o")
                            for j in range(NJ):
                                s_ps = ps_att.tile([128, S], f32, tag="s", name="s")
                                for n in range(NN):
                                    nc.tensor.matmul(
                                        s_ps[:, CH * n:CH * (n + 1)],
                                        kz_[h][:, 128 * j:128 * (j + 1)],
                                        qp[h][:, CH * n:CH * (n + 1)],
                                        start=True, stop=True)
                                ex = expp.tile([128, S], bf16, tag="exp", name="exp")
                                nc.scalar.activation(ex, s_ps, AF.Exp, scale=INV)
                                for n in range(NN):
                                    nc.tensor.matmul(
                                        o_ps[:, CH * n:CH * (n + 1)],
                                        va[j][:, h, :],
                                        ex[:, CH * n:CH * (n + 1)],
                                        start=(j == 0), stop=(j == NJ - 1))
                            os_ = osp.tile([65, S], f32, tag=f"os{t}_{half}",
                                           name=f"os{t}_{half}")
                            nc.vector.tensor_copy(os_, o_ps)
                            nc.sync.dma_start(
                                out=sp[2 * half + t - 1:2 * half + t, :],
                                in_=os_[64:65, :])
                            os_c[t] = os_
                        os_t[half] = (os_c[1], os_c[2])
                    emit_combine(sp, os_t, p)

            # ---------- phase 3: stats, groupnorm, gate, output ----------
            with tc.tile_pool(name="tailp", bufs=1) as tailp, \
                 tc.tile_pool(name="oq", bufs=3) as oqp, \
                 tc.tile_pool(name="ps_tail", bufs=2, space="PSUM") as ps_tail:

                tot = tailp.tile([64, 2], f32, tag="tot", name="tot")
                nc.vector.tensor_reduce(
                    tot, sumcol.rearrange("p (t h) -> p t h", h=8),
                    axis=AX.X, op=OP.add)
                # bias-C (bv) corrections to the raw-Y stats
                csc = tailp.tile([64, 8], f32, tag="csc", name="csc")
                nc.vector.tensor_mul(csc, cc, sumcol[:, 0:8])
                cy64 = tailp.tile([64, 1], f32, tag="cy64", name="cy64")
                nc.vector.tensor_reduce(cy64, csc, axis=AX.X, op=OP.add)
                nc.vector.tensor_mul(csc, cc, cc)
                csq64 = tailp.tile([64, 1], f32, tag="csq64", name="csq64")
                nc.vector.tensor_reduce(csq64, csc, axis=AX.X, op=OP.add)
                csum64 = tailp.tile([64, 1], f32, tag="csum64", name="csum64")
                nc.vector.tensor_reduce(csum64, cc, axis=AX.X, op=OP.add)
                tot2 = tailp.tile([64, 2], f32, tag="tot2", name="tot2")
                stt(tot2[:, 0:1], csum64, float(S), tot[:, 0:1], OP.mult, OP.add)
                stt(tot2[:, 1:2], cy64, 2.0, tot[:, 1:2], OP.mult, OP.add)
                stt(tot2[:, 1:2], csq64, float(S), tot2[:, 1:2], OP.mult, OP.add)

                ms_ps = ps_tail.tile([64, 2], f32, tag="ms", name="ms")
                nc.tensor.matmul(ms_ps, ind2, tot2, start=True, stop=True)
                mean64 = tailp.tile([64, 1], f32, tag="mean64", name="mean64")
                ts_(mean64, ms_ps[:, 0:1], 1.0 / CNT, None, OP.mult)
                e264 = tailp.tile([64, 1], f32, tag="e264", name="e264")
                ts_(e264, ms_ps[:, 1:2], 1.0 / CNT, None, OP.mult)
                nm2 = tailp.tile([64, 1], f32, tag="nm2", name="nm2")
                ts_(nm2, mean64, mean64, -1.0, OP.mult, OP.mult)
                veps = tailp.tile([64, 1], f32, tag="veps", name="veps")
                stt(veps, nm2, EPS, e264, OP.add, OP.add)
                sd = tailp.tile([64, 1], f32, tag="sd", name="sd")
                nc.scalar.activation(sd, veps, AF.Sqrt)
                rsd = tailp.tile([64, 1], f32, tag="rsd", name="rsd")
                nc.vector.reciprocal(rsd, sd)
                # one Newton step for rsqrt accuracy (ACT sqrt is loose)
                rr = tailp.tile([64, 1], f32, tag="rr", name="rr")
                nc.vector.tensor_mul(rr, rsd, rsd)
                nc.vector.tensor_mul(rr, rr, veps)
                ts_(rr, rr, -0.5, 1.5, OP.mult, OP.add)
                rstd = tailp.tile([64, 1], f32, tag="rstd", name="rstd")
                nc.vector.tensor_mul(rstd, rsd, rr)

                a64 = tailp.tile([64, 1], f32, tag="a64", name="a64")
                ts_(a64, rstd, gamma_c, halfli, OP.mult, OP.mult)
                cm = tailp.tile([64, 8], f32, tag="cm", name="cm")
                ts_(cm, cc, mean64, None, OP.subtract)
                ball = tailp.tile([64, 8], f32, tag="ball", name="ball")
                ts_(ball, cm, a64, bb64, OP.mult, OP.add)

                for p in range(4):
                    for half in range(2):
                        h = 2 * p + half
                        rows = ypair[p][64 * half:64 * half + 64, :]
                        ts_(rows, rows, a64, ball[:, h:h + 1], OP.mult, OP.add)
                    stt(ypair[p], th_t[p], 1.0, ypair[p], OP.add, OP.mult)

                for c in range(NJ):
                    tp_o = ps_tail.tile([128, 512], f32, tag="tp_out", name="tp_out")
                    for p in range(4):
                        nc.tensor.transpose(
                            tp_o[:, 128 * p:128 * (p + 1)],
                            ypair[p][:, 128 * c:128 * (c + 1)], ident)
                    oq = oqp.tile([128, 512], f32, tag="oq", name="oq")
                    nc.vector.tensor_copy(oq, tp_o)
                    nc.sync.dma_start(out=out_d[128 * c:128 * (c + 1), :], in_=oq)

    nc.finalize()
    return nc


_CACHE = {}


def _get_nc():
    if "nc" not in _CACHE:
        _CACHE["nc"] = build_nc(S_FULL)
    return _CACHE["nc"]


def run(inputs, trace=False, tmpdir=None):
    from concourse.bass_utils import run_bass_kernel_spmd
    nc = _get_nc()
    arrs = {k: np.asarray(v, dtype=np.float32) for k, v in inputs.items()}
    shared = {k: np.ascontiguousarray(arrs[k]) for k in
              ("Wq", "bq", "Wk", "bk", "Wv", "bv", "gamma", "beta",
               "lam", "lambda_init")}
    in_maps = []
    for i in range(B):
        m = dict(shared)
        m["query"] = np.ascontiguousarray(arrs["query"][i])
        m["key"] = np.ascontiguousarray(arrs["key"][i])
        m["values"] = np.ascontiguousarray(arrs["values"][i])
        in_maps.append(m)
    res = run_bass_kernel_spmd(nc, in_maps, core_ids=list(range(B)),
                               trace=trace, tmpdir=tmpdir)
    out = np.stack([res.results[i]["out"] for i in range(B)], axis=0)
    return out.astype(np.float32), res


def kernel(**inputs):
    out, _ = run(inputs)
    return out



# revision 9
# speedup vs baseline: 1.0221x; 1.0221x over previous
# Differential multi-head attention (dual softmax + GroupNorm + sigmoid gating)
# for Trainium2, batch-parallel across 8 NeuronCores (one batch row per core).
#
# Per-core math (batch b):
#   q = query @ Wq + bq -> per head: q1, q2, gate (each S x 64)
#   k = key   @ Wk + bk -> per head: k1, k2
#   v = values@ Wv + bv -> per head: v (S x 64)
#   attn = softmax(q1 k1^T / 8) - lam * softmax(q2 k2^T / 8)
#   out  = GroupNorm_{8 groups over d, reduced over (S, heads, d-in-group)}(attn @ v)
#   out  = out * (1 - lambda_init) * sigmoid(gate)
#
# Layout strategy:
#  - Host pre-packs x^T and all weights as bf16 (layout marshalling only; the
#    math runs on device).  q/k projections are d-major (q1/q2 resp. k1/k2 in
#    complementary 64-partition halves of one [128,S] tile); score matmuls are
#    K=64 at PE row-tiles 0/64 (no zero padding needed).
#  - "Flipped" o-matmul: stationary = exp-score slice [128k x 128q], moving =
#    v_aug [128k x 65] -> out is q-major [128q, 65] with the exp row-sum in
#    column 64 (ones column in v_aug).  This halves PE time vs the d-major
#    o-matmul, makes softmax normalization a per-partition scalar multiply,
#    and leaves y in the exact output layout (no final transposes).
#  - GroupNorm stats via ones-column f32r matmuls (Sum y, Sum y^2 over seq on
#    partitions), group-reduced on DVE, rsqrt via sqrt+NR, then the per-column
#    affine A,B is expanded back to [128,512] with tiny K=1/K=8 matmuls.
#  - ACT runs (nearly) only the 128 [128,1024] exp instructions + 8 tanh; all
#    projection epilogues run on DVE/Pool.  PSUM: s double-buffered (4 banks),
#    o manually packed into 3 banks (7+7+2 groups of 65 cols), proj 1 bank.

import numpy as np

B, S, H, D = 8, 1024, 8, 64
DM = H * D  # 512
NJ = S // 128   # key 128-tiles
NQ = S // 128   # query 128-blocks
EPS = 1e-3
INV = 0.125
CNT = float(S * H)  # groupnorm count per group = S * H * (D//H) / ... = 1024*8


def build_nc():
    import concourse.bacc as bacc
    import concourse.tile as tile
    from concourse import mybir

    f32 = mybir.dt.float32
    f32r = mybir.dt.float32r
    bf16 = mybir.dt.bfloat16
    AF = mybir.ActivationFunctionType
    OP = mybir.AluOpType
    AX = mybir.AxisListType

    nc = bacc.Bacc(target_bir_lowering=False)
    xq_d = nc.dram_tensor("xq", [128, 4 * S], bf16, kind="ExternalInput")
    xk_d = nc.dram_tensor("xk", [128, 4 * S], bf16, kind="ExternalInput")
    xv_d = nc.dram_tensor("xv", [128, 4 * S], bf16, kind="ExternalInput")
    wq_d = nc.dram_tensor("wq", [128, 4 * 1536], bf16, kind="ExternalInput")
    wk_d = nc.dram_tensor("wk", [128, 4 * 1024], bf16, kind="ExternalInput")
    wv_d = nc.dram_tensor("wv", [128, 4 * 512], bf16, kind="ExternalInput")
    wg_d = nc.dram_tensor("wg", [128, 4 * 512], bf16, kind="ExternalInput")
    bqk_d = nc.dram_tensor("bqk", [128, 16], f32, kind="ExternalInput")
    rows_d = nc.dram_tensor("rows", [1, 2048], f32, kind="ExternalInput")
    rowsb_d = nc.dram_tensor("rowsb", [1, 1024], bf16, kind="ExternalInput")
    g8_d = nc.dram_tensor("g8", [8, 512], f32, kind="ExternalInput")
    negl_d = nc.dram_tensor("negl", [1], f32, kind="ExternalInput")
    out_d = nc.dram_tensor("out", [S, DM], f32, kind="ExternalOutput")

    ts_ = nc.vector.tensor_scalar
    stt = nc.vector.scalar_tensor_tensor
    gts_ = nc.gpsimd.tensor_scalar
    gstt = nc.gpsimd.scalar_tensor_tensor

    # (t, qb) accumulation group -> (o-bank index, column offset); 65 cols per
    # group (64 d + 1 ones-sum), packed 7 + 7 + 2 into three psum banks.
    def o_loc(t, qb):
        g = t * 8 + qb
        if g < 7:
            return 0, 65 * g
        if g < 14:
            return 1, 65 * (g - 7)
        return 2, 65 * (g - 14)

    with tile.TileContext(nc) as tc:
        with tc.tile_pool(name="persist", bufs=1) as pp:
            # ---------- persistent SBUF ----------
            xq_t = pp.tile([128, 4 * S], bf16, tag="xq_t", name="xq_t")
            xk_t = pp.tile([128, 4 * S], bf16, tag="xk_t", name="xk_t")
            xv_t = pp.tile([128, 4 * S], bf16, tag="xv_t", name="xv_t")
            wq_t = pp.tile([128, 4 * 1536], bf16, tag="wq_t", name="wq_t")
            wk_t = pp.tile([128, 4 * 1024], bf16, tag="wk_t", name="wk_t")
            wv_t = pp.tile([128, 4 * 512], bf16, tag="wv_t", name="wv_t")
            wg_t = pp.tile([128, 4 * 512], bf16, tag="wg_t", name="wg_t")
            bqk_t = pp.tile([128, 16], f32, tag="bqk_t", name="bqk_t")
            rows_t = pp.tile([1, 2048], f32, tag="rows_t", name="rows_t")
            rowsb_t = pp.tile([1, 1024], bf16, tag="rowsb_t", name="rowsb_t")
            g8_t = pp.tile([8, 512], f32, tag="g8_t", name="g8_t")
            negl_t = pp.tile([128, 1], f32, tag="negl_t", name="negl_t")
            ones_c = pp.tile([128, 1], bf16, tag="ones_c", name="ones_c")
            ones_r = pp.tile([1, 128], bf16, tag="ones_r", name="ones_r")
            ones_rf = pp.tile([1, 128], f32, tag="ones_rf", name="ones_rf")
            one1 = pp.tile([1, 1], f32, tag="one1", name="one1")
            qp = [pp.tile([128, S], bf16, tag=f"qp{h}", name=f"qp{h}") for h in range(8)]
            kp = [pp.tile([128, S], bf16, tag=f"kp{h}", name=f"kp{h}") for h in range(8)]
            va = [pp.tile([128, 8, 65], bf16, tag=f"va{j}", name=f"va{j}") for j in range(NJ)]
            th = [pp.tile([128, 512], f32, tag=f"th{q}", name=f"th{q}") for q in range(NQ)]
            yy = [pp.tile([128, 512], f32, tag=f"yy{q}", name=f"yy{q}") for q in range(NQ)]
            mrst = pp.tile([8, 2], f32, tag="mrst", name="mrst")

            # ---------- DMAs (3 HWDGE queues + gpsimd for small consts) ----
            nc.sync.dma_start(out=xq_t, in_=xq_d[:, :])
            nc.sync.dma_start(out=xk_t, in_=xk_d[:, :])
            nc.sync.dma_start(out=xv_t, in_=xv_d[:, :])
            nc.scalar.dma_start(out=wq_t, in_=wq_d[:, :])
            nc.scalar.dma_start(out=wk_t, in_=wk_d[:, :])
            nc.scalar.dma_start(out=wv_t, in_=wv_d[:, :])
            nc.scalar.dma_start(out=wg_t, in_=wg_d[:, :])
            nc.gpsimd.dma_start(out=bqk_t, in_=bqk_d[:, :])
            nc.gpsimd.dma_start(out=rows_t, in_=rows_d[:, :])
            nc.gpsimd.dma_start(out=rowsb_t, in_=rowsb_d[:, :])
            nc.gpsimd.dma_start(out=g8_t, in_=g8_d[:, :])
            nc.gpsimd.dma_start(out=negl_t, in_=negl_d[:].to_broadcast([128, 1]))
            nc.gpsimd.memset(ones_c, 1.0)
            nc.gpsimd.memset(ones_r, 1.0)
            nc.gpsimd.memset(ones_rf, 1.0)
            nc.gpsimd.memset(one1, 1.0)
            for j in range(NJ):
                nc.gpsimd.memset(va[j][:, :, 64:65], 1.0)

            ghl_r = rows_t[:, 0:512]
            bhl_r = rows_t[:, 512:1024]
            gb_r = rowsb_t[:, 0:512]
            vb_r = rowsb_t[:, 512:1024]

            with tc.tile_pool(name="ps_proj", bufs=1, space="PSUM") as ps_proj, \
                 tc.tile_pool(name="ps_s", bufs=2, space="PSUM") as ps_s, \
                 tc.tile_pool(name="ps_o", bufs=1, space="PSUM") as ps_o, \
                 tc.tile_pool(name="expp", bufs=3) as expp, \
                 tc.tile_pool(name="rscp", bufs=2) as rscp:

                # ---------- projection emitters ----------
                def qproj(h, eng_i):
                    eng = ts_
                    for n in range(2):
                        ps = ps_proj.tile([128, 512], f32, tag="proj", name="proj")
                        for r in range(4):
                            nc.tensor.matmul(
                                ps, wq_t[:, 1536 * r + 192 * h:1536 * r + 192 * h + 128],
                                xq_t[:, S * r + 512 * n:S * r + 512 * (n + 1)],
                                start=(r == 0), stop=(r == 3))
                        eng(qp[h][:, 512 * n:512 * (n + 1)], ps,
                            bqk_t[:, h:h + 1], None, OP.add)

                def kproj(h, eng_i):
                    eng = ts_
                    for n in range(2):
                        ps = ps_proj.tile([128, 512], f32, tag="proj", name="proj")
                        for r in range(4):
                            nc.tensor.matmul(
                                ps, wk_t[:, 1024 * r + 128 * h:1024 * r + 128 * h + 128],
                                xk_t[:, S * r + 512 * n:S * r + 512 * (n + 1)],
                                start=(r == 0), stop=(r == 3))
                        eng(kp[h][:, 512 * n:512 * (n + 1)], ps,
                            bqk_t[:, 8 + h:9 + h], None, OP.add)

                def vproj(j):
                    ps = ps_proj.tile([128, 512], f32, tag="proj", name="proj")
                    for r in range(4):
                        nc.tensor.matmul(
                            ps, xv_t[:, S * r + 128 * j:S * r + 128 * (j + 1)],
                            wv_t[:, 512 * r:512 * (r + 1)],
                            start=(r == 0), stop=False)
                    nc.tensor.matmul(ps, ones_r[:], vb_r, start=False, stop=True)
                    nc.vector.tensor_copy(
                        va[j][:, :, 0:64], ps.rearrange("p (h d) -> p h d", d=64))

                def gproj(q):
                    ps = ps_proj.tile([128, 512], f32, tag="proj", name="proj")
                    for r in range(4):
                        nc.tensor.matmul(
                            ps, xq_t[:, S * r + 128 * q:S * r + 128 * (q + 1)],
                            wg_t[:, 512 * r:512 * (r + 1)],
                            start=(r == 0), stop=False)
                    nc.tensor.matmul(ps, ones_r[:], gb_r, start=False, stop=True)
                    nc.scalar.activation(th[q], ps, AF.Tanh, scale=0.5)

                # ---------- combine: y[:, 64h:64h+64] = o1*r1 - lam*o2*r2 --
                def combine(h, ot):
                    rsc = rscp.tile([128, 16], f32, tag="rsc", name="rsc")
                    nc.vector.reciprocal(
                        rsc[:, 0:7],
                        ot[0][:, 0:455].rearrange("p (g c) -> p g c", c=65)[:, :, 64])
                    nc.vector.reciprocal(
                        rsc[:, 7:14],
                        ot[1][:, 0:455].rearrange("p (g c) -> p g c", c=65)[:, :, 64])
                    nc.vector.reciprocal(
                        rsc[:, 14:16],
                        ot[2][:, 0:130].rearrange("p (g c) -> p g c", c=65)[:, :, 64])
                    ts_(rsc[:, 8:16], rsc[:, 8:16], negl_t, None, OP.mult)
                    for qb in range(8):
                        b1, c1 = o_loc(0, qb)
                        b2, c2 = o_loc(1, qb)
                        ydst = yy[qb][:, 64 * h:64 * h + 64]
                        ts_(ydst, ot[b1][:, c1:c1 + 64],
                            rsc[:, qb:qb + 1], None, OP.mult)
                        stt(ydst, ot[b2][:, c2:c2 + 64],
                            rsc[:, 8 + qb:9 + qb], ydst, OP.mult, OP.add)

                # ---------- emission ----------
                qproj(0, 0)
                qproj(1, 1)
                kproj(0, 0)
                kproj(1, 1)
                for j in range(NJ):
                    vproj(j)
                for q in range(NQ):
                    gproj(q)

                for h in range(8):
                    if h + 2 < 8:
                        qproj(h + 2, 0)
                        kproj(h + 2, 1)
                    ot = [ps_o.tile([128, 512], f32, tag=f"o{i}", name=f"o{i}")
                          for i in range(3)]
                    for t in range(2):
                        for j in range(NJ):
                            s_t = ps_s.tile([128, S], f32, tag="s", name="s")
                            for n in range(2):
                                nc.tensor.matmul(
                                    s_t[:, 512 * n:512 * (n + 1)],
                                    kp[h][64 * t:64 * t + 64, 128 * j:128 * (j + 1)],
                                    qp[h][64 * t:64 * t + 64, 512 * n:512 * (n + 1)],
                                    start=True, stop=True)
                            ex = expp.tile([128, S], bf16, tag="ex", name="ex")
                            nc.scalar.activation(ex, s_t, AF.Exp, scale=INV)
                            for qb in range(NQ):
                                bi, c0 = o_loc(t, qb)
                                # start only on the FIRST matmul into each
                                # bank this head: start_tensor_calc zeroes
                                # the whole bank, so later groups must
                                # accumulate onto the zeroed regions.
                                first = (j == 0) and (t, qb) in ((0, 0), (0, 7), (1, 6))
                                nc.tensor.matmul(
                                    ot[bi][:, c0:c0 + 65],
                                    ex[:, 128 * qb:128 * (qb + 1)],
                                    va[j][:, h, :],
                                    start=first, stop=(j == NJ - 1),
                                    skip_group_check=True)
                    combine(h, ot)

            # ---------- tail: GroupNorm stats + affine + gate + output ----
            with tc.tile_pool(name="ps_tail", bufs=1, space="PSUM") as ps_t, \
                 tc.tile_pool(name="y2p", bufs=2) as y2p, \
                 tc.tile_pool(name="oqp", bufs=3) as oqp, \
                 tc.tile_pool(name="tsb", bufs=1) as tsb:

                sy = ps_t.tile([1, 512], f32, tag="sy", name="sy")
                sy2 = ps_t.tile([1, 512], f32, tag="sy2", name="sy2")
                mcol = ps_t.tile([8, 2], f32, tag="mcol", name="mcol")
                rxA = ps_t.tile([1, 512], f32, tag="rxA", name="rxA")
                rxB = ps_t.tile([1, 512], f32, tag="rxB", name="rxB")
                ab = ps_t.tile([128, 1024], f32, tag="ab", name="ab")

                for qb in range(NQ):
                    ybf = y2p.tile([128, 512], bf16, tag="ybf", name="ybf")
                    nc.vector.tensor_copy(ybf, yy[qb])
                    nc.tensor.matmul(sy, ones_c[:], ybf,
                                     start=(qb == 0), stop=(qb == NQ - 1))
                    y2 = y2p.tile([128, 512], bf16, tag="y2", name="y2")
                    nc.gpsimd.tensor_mul(y2, yy[qb], yy[qb])
                    nc.tensor.matmul(sy2, ones_c[:], y2,
                                     start=(qb == 0), stop=(qb == NQ - 1))

                gsum = tsb.tile([1, 8], f32, tag="gsum", name="gsum")
                g2 = tsb.tile([1, 8], f32, tag="g2", name="g2")
                nc.vector.tensor_reduce(
                    gsum, sy.rearrange("o (h g e) -> o g h e", h=8, g=8),
                    axis=AX.XY, op=OP.add)
                nc.vector.tensor_reduce(
                    g2, sy2.rearrange("o (h g e) -> o g h e", h=8, g=8),
                    axis=AX.XY, op=OP.add)
                nc.tensor.matmul(mcol[:, 0:1], gsum, one1, start=True, stop=False,
                                 skip_group_check=True)
                nc.tensor.matmul(mcol[:, 1:2], g2, one1, start=False, stop=True,
                                 skip_group_check=True)

                e2t = tsb.tile([8, 1], f32, tag="e2t", name="e2t")
                nm = tsb.tile([8, 1], f32, tag="nm", name="nm")
                veps = tsb.tile([8, 1], f32, tag="veps", name="veps")
                sd = tsb.tile([8, 1], f32, tag="sd", name="sd")
                rsd = tsb.tile([8, 1], f32, tag="rsd", name="rsd")
                rr = tsb.tile([8, 1], f32, tag="rr", name="rr")
                ts_(mrst[:, 1:2], mcol[:, 0:1], 1.0 / float(S * H * 8), None, OP.mult)
                ts_(e2t, mcol[:, 1:2], 1.0 / float(S * H * 8), None, OP.mult)
                ts_(nm, mrst[:, 1:2], mrst[:, 1:2], -1.0, OP.mult, OP.mult)
                stt(veps, nm, EPS, e2t, OP.add, OP.add)
                nc.scalar.activation(sd, veps, AF.Sqrt)
                nc.vector.reciprocal(rsd, sd)
                nc.vector.tensor_mul(rr, rsd, rsd)
                nc.vector.tensor_mul(rr, rr, veps)
                ts_(rr, rr, -0.5, 1.5, OP.mult, OP.add)
                nc.vector.tensor_mul(mrst[:, 0:1], rsd, rr)

                nc.tensor.matmul(rxA, mrst[:, 0:1], g8_t[:, :], start=True, stop=True)
                nc.tensor.matmul(rxB, mrst[:, 1:2], g8_t[:, :], start=True, stop=True)
                arow = tsb.tile([1, 512], f32, tag="arow", name="arow")
                btmp = tsb.tile([1, 512], f32, tag="btmp", name="btmp")
                brow = tsb.tile([1, 512], f32, tag="brow", name="brow")
                nc.vector.tensor_mul(arow, ghl_r, rxA)
                nc.vector.tensor_mul(btmp, rxB, arow)
                nc.vector.tensor_sub(brow, bhl_r, btmp)
                nc.tensor.matmul(ab[:, 0:512], ones_rf[:], arow,
                                 start=True, stop=True)
                nc.tensor.matmul(ab[:, 512:1024], ones_rf[:], brow,
                                 start=True, stop=True)

                ab_sb = tsb.tile([128, 1024], f32, tag="ab_sb", name="ab_sb")
                nc.vector.tensor_copy(ab_sb[:, 0:512], ab[:, 0:512])
                nc.vector.tensor_copy(ab_sb[:, 512:1024], ab[:, 512:1024])
                for qb in range(NQ):
                    oq = oqp.tile([128, 512], f32, tag="oq", name="oq")
                    if qb % 2 == 0:
                        nc.vector.tensor_mul(oq, yy[qb], ab_sb[:, 0:512])
                        nc.vector.tensor_add(oq, oq, ab_sb[:, 512:1024])
                        stt(oq, th[qb], 1.0, oq, OP.add, OP.mult)
                    else:
                        nc.gpsimd.tensor_mul(oq, yy[qb], ab_sb[:, 0:512])
                        nc.gpsimd.tensor_add(oq, oq, ab_sb[:, 512:1024])
                        stt(oq, th[qb], 1.0, oq, OP.add, OP.mult)
                    nc.sync.dma_start(out=out_d[128 * qb:128 * (qb + 1), :], in_=oq)

    nc.finalize()
    return nc


_CACHE = {}


def _get_nc():
    if "nc" not in _CACHE:
        _CACHE["nc"] = build_nc()
    return _CACHE["nc"]


def _host_prep(arrs):
    """Pack weights/biases into device layouts (bf16 x^T chunks etc.)."""
    from ml_dtypes import bfloat16 as bf

    def rpack(w):  # [512, C] -> [128, 4*C] with [p, C*r + c] = w[128r + p, c]
        c = w.shape[1]
        return np.ascontiguousarray(
            w.reshape(4, 128, c).transpose(1, 0, 2).reshape(128, 4 * c)).astype(bf)

    wq, wk, wv = arrs["Wq"], arrs["Wk"], arrs["Wv"]
    wg = np.ascontiguousarray(wq.reshape(DM, 8, 192)[:, :, 128:].reshape(DM, 512))
    bq, bk, bv = arrs["bq"], arrs["bk"], arrs["bv"]
    lam = float(arrs["lam"][0])
    li = float(arrs["lambda_init"][0])
    hl = 0.5 * (1.0 - li)

    bqk = np.zeros((128, 16), np.float32)
    for h in range(8):
        bqk[:, h] = bq[192 * h:192 * h + 128]
        bqk[:, 8 + h] = bk[128 * h:128 * h + 128]
    gb = bq.reshape(8, 192)[:, 128:].reshape(512)
    rows = np.concatenate([
        np.tile(arrs["gamma"], 8) * hl,
        np.tile(arrs["beta"], 8) * hl,
        gb, bv]).astype(np.float32).reshape(1, 2048)
    g8 = np.zeros((8, 512), np.float32)
    cols = np.arange(512)
    g8[(cols % 64) // 8, cols] = 1.0

    rowsb = np.concatenate([gb, bv]).astype(bf).reshape(1, 1024)
    shared = {
        "wq": rpack(wq), "wk": rpack(wk), "wv": rpack(wv), "wg": rpack(wg),
        "bqk": np.ascontiguousarray(bqk), "rows": rows, "rowsb": rowsb,
        "g8": g8, "negl": np.array([-lam], np.float32),
    }
    in_maps = []
    for i in range(B):
        m = dict(shared)
        for nm, key in (("xq", "query"), ("xk", "key"), ("xv", "values")):
            m[nm] = rpack(np.ascontiguousarray(arrs[key][i].T))
        in_maps.append(m)
    return in_maps


def run(inputs, trace=False, tmpdir=None):
    from concourse.bass_utils import run_bass_kernel_spmd
    nc = _get_nc()
    arrs = {k: np.asarray(v, dtype=np.float32) for k, v in inputs.items()}
    in_maps = _host_prep(arrs)
    res = run_bass_kernel_spmd(nc, in_maps, core_ids=list(range(B)),
                               trace=trace, tmpdir=tmpdir)
    out = np.stack([res.results[i]["out"] for i in range(B)], axis=0)
    return out.astype(np.float32), res


def kernel(**inputs):
    out, _ = run(inputs)
    return out
